# revision 22
# baseline (speedup 1.0000x reference)
"""Trainium2 Bass kernel for nn_Attention_48395691491550 (sparse hierarchical attention).

Sharding: head-parallel — 8 heads across 8 NeuronCores (B=1).
Two device launches with a tiny host control-plane step between them:
  L1: per-head QKV projection (+ keyframe diag scores for patch selection)
  host: softmax/mean/top-k over [4,256] scores -> selected patch token ids
  L2: dma_gather of selected K/V, transposed-score attention (exp on ScalarE),
      cluster-score AllReduce-max -> neighbor mask folded into V, AllToAll of
      per-head outputs, per-core token-slice output projection.
"""

import numpy as np
import ml_dtypes

import concourse.bass as bass
import concourse.bacc as bacc
import concourse.tile as tile
import concourse.mybir as mybir
import concourse.bass_isa as bass_isa
from concourse.bass_utils import run_bass_kernel_spmd

BF16 = ml_dtypes.bfloat16
F32 = mybir.dt.float32
BF = mybir.dt.bfloat16
F32R = mybir.dt.float32r
I16 = mybir.dt.int16
AF = mybir.ActivationFunctionType

# Problem constants (hardcoded per spec)
H = 8
N = 4096
C = 512
D = 64          # head dim
K = 4           # clusters
F = 4           # frames per cluster
P = 256         # patches (tokens) per frame
TK = 128        # top-k patches per cluster (P * 0.5)
NSEL = K * F * TK   # 2048 selected keys for main attention
NKF = K * P         # 1024 keyframe tokens
SCALE = D ** -0.5
N_CORES = 8
TOK0 = N // N_CORES  # 512 tokens per core own the output projection

TRACE = False          # set by test harness for profiling
LAST_EXEC_NS = {}      # launch name -> exec ns (filled when TRACE)


# --------------------------------------------------------------------------- #
# Launch 1: QKV projection + keyframe diag scores
# --------------------------------------------------------------------------- #
def _build_l1():
    nc = bacc.Bacc("TRN2", target_bir_lowering=False, debug=False,
                   num_devices=N_CORES)
    xTb = nc.dram_tensor("xTb", [C + 1, N], BF, kind="ExternalInput")
    wqT = nc.dram_tensor("wqT", [C, D], BF, kind="ExternalInput")
    wkvT = nc.dram_tensor("wkvT", [C, 2 * D], BF, kind="ExternalInput")
    bq = nc.dram_tensor("bq", [D, 1], F32, kind="ExternalInput")
    bk = nc.dram_tensor("bk", [D, 1], F32, kind="ExternalInput")
    bkv = nc.dram_tensor("bkv", [1, 2 * D], BF, kind="ExternalInput")

    qT_d = nc.dram_tensor("qT_d", [D, N], BF, kind="ExternalOutput")
    KV_d = nc.dram_tensor("KV_d", [N, 2 * D], BF, kind="ExternalOutput")
    diag_d = nc.dram_tensor("diag_d", [1, NKF], F32, kind="ExternalOutput")

    NT = N // 128        # 32 token tiles
    NCH = N // 512       # 8 chunks
    KT = C // 128        # 4 contraction tiles

    with tile.TileContext(nc) as tc:
        with (
            tc.tile_pool(name="sb", bufs=1) as sb,
            tc.tile_pool(name="pipe", bufs=4) as pipe,
            tc.tile_pool(name="ps", bufs=2, space="PSUM") as ps,
            tc.tile_pool(name="pskv", bufs=2, space="PSUM") as pskv,
            tc.tile_pool(name="psdg", bufs=1, space="PSUM") as psdg,
        ):
            xt = []
            for i in range(KT):
                t = sb.tile([128, N], BF, tag=f"xt{i}")
                xt.append(t)
            for half in range(2):
                for i in range(KT):
                    eng = nc.sync if i % 2 == 0 else nc.scalar
                    eng.dma_start(
                        xt[i][:, half * 2048:(half + 1) * 2048],
                        xTb[i * 128:(i + 1) * 128,
                            half * 2048:(half + 1) * 2048])
            xones = sb.tile([1, N], BF, tag="xones")
            nc.sync.dma_start(xones[:], xTb[C:C + 1, :])
            wq_sb = sb.tile([128, KT, D], BF, tag="wq")
            nc.gpsimd.dma_start(wq_sb[:],
                                wqT.rearrange("(k p) d -> p k d", p=128))
            wkv_sb = sb.tile([128, KT, 2 * D], BF, tag="wkv")
            nc.gpsimd.dma_start(wkv_sb[:],
                                wkvT.rearrange("(k p) d -> p k d", p=128))
            bq_sb = sb.tile([D, 1], F32, tag="bq")
            nc.gpsimd.dma_start(bq_sb[:], bq[:])
            bk_sb = sb.tile([D, 1], F32, tag="bk")
            nc.gpsimd.dma_start(bk_sb[:], bk[:])
            bkv_sb = sb.tile([1, 2 * D], BF, tag="bkv")
            nc.gpsimd.dma_start(bkv_sb[:], bkv[:])

            # ---- q^T + K|V per 512-token chunk (pipelines with loads) ----
            qT_sb = sb.tile([D, N], BF, tag="qT")
            KV_d_v = KV_d.rearrange("(t p) d -> p t d", p=128)
            for ch in range(NCH):
                pq = ps.tile([D, 512], F32, tag="pq")
                for kt in range(KT):
                    nc.tensor.matmul(
                        pq[:], wq_sb[:, kt, :],
                        xt[kt][:, ch * 512:(ch + 1) * 512],
                        start=(kt == 0), stop=(kt == KT - 1))
                nc.scalar.activation(qT_sb[:, ch * 512:(ch + 1) * 512], pq[:],
                                     AF.Identity, bias=bq_sb[:])
                kv_sb = pipe.tile([128, 4, 2 * D], BF, tag="kv")
                for s in range(4):
                    tt = ch * 4 + s
                    pkv = pskv.tile([128, 2 * D], F32, tag="pkv")
                    for kt in range(KT):
                        nc.tensor.matmul(
                            pkv[:], xt[kt][:, tt * 128:(tt + 1) * 128],
                            wkv_sb[:, kt, :],
                            start=(kt == 0), stop=False)
                    nc.tensor.matmul(pkv[:],
                                     xones[:, tt * 128:(tt + 1) * 128],
                                     bkv_sb[:], start=False, stop=True)
                    nc.vector.tensor_copy(kv_sb[:, s, :], pkv[:])
                nc.sync.dma_start(KV_d_v[:, ch * 4:(ch + 1) * 4, :], kv_sb[:])
            nc.sync.dma_start(qT_d[:], qT_sb[:])

            # ---- keyframe k^T [64, 1024] + diag scores ----
            # kf tokens: first 256 of each 1024-block (clusters = arange)
            kfkT = sb.tile([D, NKF], BF, tag="kfkT")
            for half in range(2):
                pk = ps.tile([D, 512], F32, tag="pkf")
                for kt in range(KT):
                    rhs = xt[kt][:, half * 2048:(half + 1) * 2048]
                    rhs = rhs.rearrange("p (b t) -> p b t", b=2)[:, :, 0:256]
                    nc.tensor.matmul(pk[:],
                                     wkv_sb[:, kt, 0:D],
                                     rhs, start=(kt == 0), stop=(kt == KT - 1))
                nc.scalar.activation(kfkT[:, half * 512:(half + 1) * 512],
                                     pk[:], AF.Identity, bias=bk_sb[:])
            # q at kf tokens: strided view of qT_sb
            qkf = qT_sb.rearrange("p (b t) -> p b t", b=K)[:, :, 0:256]
            prod = sb.tile([D, NKF], BF, tag="prod")
            nc.vector.tensor_mul(
                prod.rearrange("p (b t) -> p b t", b=K)[:], qkf,
                kfkT.rearrange("p (b t) -> p b t", b=K)[:])
            ones64 = sb.tile([D, 1], BF, tag="ones64")
            nc.vector.memset(ones64[:], 1.0)
            pdiag = psdg.tile([1, NKF], F32, tag="pdiag")
            for half in range(2):
                nc.tensor.matmul(pdiag[:, half * 512:(half + 1) * 512],
                                 ones64[:],
                                 prod[:, half * 512:(half + 1) * 512],
                                 start=True, stop=True)
            diag_sb = sb.tile([1, NKF], F32, tag="diag")
            nc.vector.tensor_copy(diag_sb[:], pdiag[:])
            nc.sync.dma_start(diag_d[:], diag_sb[:])

    nc.compile()
    return nc


# --------------------------------------------------------------------------- #
# Launch 2: gathered sparse attention + projection
# --------------------------------------------------------------------------- #
def _build_l2():
    nc = bacc.Bacc("TRN2", target_bir_lowering=False, debug=False,
                   num_devices=N_CORES)
    qT_i = nc.dram_tensor("qT_i", [D, N], BF, kind="ExternalInput")
    KV_i = nc.dram_tensor("KV_i", [N, 2 * D], BF, kind="ExternalInput")
    G_i = nc.dram_tensor("G_i", [N, 128], BF, kind="ExternalInput")
    qselm_i = nc.dram_tensor("qselm_i", [NKF, 1], F32, kind="ExternalInput")
    projTb_i = nc.dram_tensor("projTb_i", [C + 1, C], BF, kind="ExternalInput")
    EB_i = nc.dram_tensor("EB_i", [H, C], F32, kind="ExternalInput")
    eye_i = nc.dram_tensor("eye_i", [K, K], F32, kind="ExternalInput")

    y_d = nc.dram_tensor("y_d", [TOK0, C], F32, kind="ExternalOutput")

    NKT = NSEL // 128     # 16 key tiles

    with tile.TileContext(nc) as tc:
        with (
            tc.tile_pool(name="sb", bufs=1) as sb,
            tc.tile_pool(name="pt", bufs=24) as pt,          # P^T pipeline
            tc.tile_pool(name="misc", bufs=4) as misc,
            tc.tile_pool(name="dram", bufs=1, space="DRAM") as dram,
            tc.tile_pool(name="psS", bufs=2, space="PSUM") as psS,
            tc.tile_pool(name="psAV", bufs=2, space="PSUM") as psAV,
            tc.tile_pool(name="psSm", bufs=2, space="PSUM") as psSm,
        ):
            # ---------------- loads + gathers ----------------
            qT2 = sb.tile([128, N], BF, tag="qT2")
            nc.scalar.dma_start(qT2[0:D, :], qT_i[:])
            nc.scalar.dma_start(qT2[D:2 * D, :], qT_i[:])
            qT = qT2[0:D, :]
            KV_sb = sb.tile([128, 32, 2 * D], BF, tag="KV_sb")
            KV_v = KV_i.rearrange("(t p) d -> p t d", p=128)
            G_sb = sb.tile([128, 32, 128], BF, tag="G_sb")
            G_v = G_i.rearrange("(t p) d -> p t d", p=128)
            for h2 in range(4):
                lo, hi = h2 * 8, (h2 + 1) * 8
                nc.sync.dma_start(KV_sb[:, lo:hi, :], KV_v[:, lo:hi, :])
                nc.scalar.dma_start(G_sb[:, lo:hi, :], G_v[:, lo:hi, :])
            qselm = sb.tile([128, NKF // 128], F32, tag="qselm")
            nc.sync.dma_start(
                qselm[:], qselm_i.rearrange("(t p) o -> p (t o)", p=128))
            projTb = []
            for i in range(4):
                t = sb.tile([128, C], BF, tag=f"projTb{i}")
                nc.gpsimd.dma_start(t[:], projTb_i[i * 128:(i + 1) * 128, :])
                projTb.append(t)
            projBias = sb.tile([1, C], BF, tag="projBias")
            nc.gpsimd.dma_start(projBias[:], projTb_i[C:C + 1, :])
            EB = sb.tile([H, C], F32, tag="EB")
            nc.gpsimd.dma_start(EB[:], EB_i[:])
            EBr = sb.tile([H, C], F32R, tag="EBr")
            nc.vector.tensor_copy(EBr[:], EB[:])
            eye4 = sb.tile([K, K], F32, tag="eye4")
            nc.gpsimd.dma_start(eye4[:], eye_i[:])

            # matmul gather: G columns are one-hot -> exact selection.
            # kpair[0:64, p, :]  = selected k^T of frame 2p
            # kpair[64:128, p, :] = selected k^T of frame 2p+1
            kpair = sb.tile([128, NKT // 2, 128], BF, tag="kpair")
            vsel = sb.tile([128, NKT, D], BF, tag="vsel")
            for p in range(NKT // 2):
                pks = psSm.tile([128, 128], F32, tag="skf")
                for i in range(2):
                    nc.tensor.matmul(pks[0:D, :],
                                     KV_sb[:, 4 * p + i, 0:D],
                                     G_sb[:, 4 * p + i, :],
                                     start=(i == 0), stop=(i == 1))
                for i in range(2):
                    nc.tensor.matmul(pks[D:2 * D, :],
                                     KV_sb[:, 4 * p + 2 + i, 0:D],
                                     G_sb[:, 4 * p + 2 + i, :],
                                     start=(i == 0), stop=(i == 1),
                                     tile_position=(0, D))
                nc.vector.tensor_copy(kpair[:, p, :], pks[:])
            for f in range(NKT):
                pvs = psSm.tile([128, D], F32, tag="skf")
                for i in range(2):
                    nc.tensor.matmul(pvs[:], G_sb[:, 2 * f + i, :],
                                     KV_sb[:, 2 * f + i, D:2 * D],
                                     start=(i == 0), stop=(i == 1))
                nc.vector.tensor_copy(vsel[:, f, :], pvs[:])

            # ---------------- proto: cluster scores ----------------
            # S_kf [1024 kf-q, 512 sel-kf-keys]; keys of cluster m at
            # kselT[:, 0, m*512 : m*512+128] (frame-local 0 = keyframe).
            qT_blk = qT.rearrange("p (b t) -> p b t", b=K)   # [64, 4, 1024]
            amax = []   # per q-tile [128, K] masked attn max
            for qt in range(NKF // 128):
                pskf = psSm.tile([128, K * 128], F32, tag="skf")
                for m in range(K):
                    nc.tensor.matmul(
                        pskf[:, m * 128:(m + 1) * 128],
                        qT_blk[:, qt // 2,
                               (qt % 2) * 128:(qt % 2) * 128 + 128],
                        kpair[0:D, 2 * m, :],
                        start=True, stop=True)
                ekf = misc.tile([128, K * 128], F32, tag="ekf")
                den = misc.tile([128, 1], F32, tag="ekf_den")
                nc.scalar.activation(ekf[:], pskf[:], AF.Exp,
                                     scale=SCALE, accum_out=den[:])
                rden = misc.tile([128, 1], F32, tag="ekf_rden")
                nc.vector.reciprocal(rden[:], den[:])
                mx = misc.tile([128, K], F32, tag="ekf_mx")
                for m in range(K):
                    nc.vector.reduce_max(
                        mx[:, m:m + 1], ekf[:, m * 128:(m + 1) * 128],
                        axis=mybir.AxisListType.X)
                am = sb.tile([128, K], F32, tag=f"amax{qt}")
                # mx * rden * qsel_mask  (two chained scalar ops)
                nc.vector.tensor_scalar(am[:], mx[:], rden[:],
                                        qselm[:, qt:qt + 1],
                                        op0=mybir.AluOpType.mult,
                                        op1=mybir.AluOpType.mult)
                amax.append(am)
            # max over q within each cluster (2 tiles of 128 -> 1 row),
            # write each cluster row into the collective input buffer.
            cin = dram.tile([K, K], F32, tag="cin")
            cout = dram.tile([K, K], F32, tag="cout")
            tall = misc.tile([128, K * K], F32, tag="csred")
            for k in range(K):
                nc.vector.tensor_max(tall[:, k * K:(k + 1) * K],
                                     amax[2 * k][:], amax[2 * k + 1][:])
            tred = misc.tile([128, K * K], F32, tag="csred2")
            nc.gpsimd.partition_all_reduce(tred[:], tall[:], 128,
                                           bass_isa.ReduceOp.max)
            nc.gpsimd.dma_start(cin.rearrange("k m -> (k m)").rearrange(
                "(o f) -> o f", o=1), tred[0:1, :])
            nc.gpsimd.collective_compute(
                "AllReduce", mybir.AluOpType.max,
                replica_groups=[list(range(N_CORES))],
                ins=[cin.opt()], outs=[cout.opt()])
            csg = sb.tile([K, K], F32, tag="csg")
            nc.gpsimd.dma_start(csg[:], cout[:])

            # neighbor mask [K, K] -> flat [1, 16] -> broadcast BC [128, 16]
            thr = sb.tile([K, 1], F32, tag="thr")
            nc.vector.reduce_sum(thr[:], csg[:], axis=mybir.AxisListType.X)
            nc.scalar.mul(thr[:], thr[:], 0.1 / K)
            nb = sb.tile([K, K], F32, tag="nb")
            nc.vector.tensor_scalar(nb[:], csg[:], thr[:], None,
                                    op0=mybir.AluOpType.is_ge)
            nc.vector.tensor_max(nb[:], nb[:], eye4[:])
            nbflat = sb.tile([1, K * K], F32, tag="nbflat")
            nc.gpsimd.dma_start(nbflat[:].rearrange("o (k m) -> k (o m)", k=K),
                                nb[:])
            ones1 = sb.tile([1, 128], BF, tag="ones1")
            nc.vector.memset(ones1[:], 1.0)
            BC = sb.tile([128, K * K], F32, tag="BC")
            nc.gpsimd.partition_broadcast(BC[:], nbflat[:])

            # V_aug per q-cluster: [128, kt, 65] = V*nb | nb
            zero4 = sb.tile([128, K], F32, tag="zero4")
            nc.vector.memset(zero4[:], 0.0)
            vaug = []
            for c1 in range(K):
                va = sb.tile([128, NKT, D + 1], BF, tag=f"vaug{c1}")
                for m in range(K):
                    col = c1 * K + m
                    nc.vector.tensor_scalar_mul(
                        va[:, 4 * m:4 * m + 4, 0:D],
                        vsel[:, 4 * m:4 * m + 4, :], BC[:, col:col + 1])
                    nc.vector.tensor_scalar(
                        va[:, 4 * m:4 * m + 4, D], zero4[:],
                        BC[:, col:col + 1], None,
                        op0=mybir.AluOpType.add)
                vaug.append(va)

            # ---------------- main attention ----------------
            # a2a_in rows [65*j : 65*(j+1)] = this head's o^T for token
            # slice j; AllToAll swaps -> rows [65*h : ...] = head h's o^T
            # for MY token slice.
            a2a_in = dram.tile([N_CORES * (D + 1), TOK0], F32, tag="a2a_in")
            a2a_out = dram.tile([N_CORES * (D + 1), TOK0], F32, tag="a2a_out")
            a2a_in_v = a2a_in.rearrange("(s p) t -> p s t", p=D + 1)
            pavs = {}

            def _emit_av(e):
                qh_, c_, kp_, pT_ = e
                pav = pavs[(qh_, c_)]
                nc.tensor.matmul(pav[:], vaug[qh_][:, 2 * kp_, :],
                                 pT_[:, 0:512],
                                 start=(kp_ == 0), stop=False)
                nc.tensor.matmul(pav[:], vaug[qh_][:, 2 * kp_ + 1, :],
                                 pT_[:, 512:1024],
                                 start=False, stop=(kp_ == NKT // 2 - 1))
                if kp_ == NKT // 2 - 1:
                    oT_sb = misc.tile([D + 1, 512], F32, tag="oT_sb")
                    nc.vector.tensor_copy(oT_sb[:], pav[:])
                    nc.sync.dma_start(a2a_in_v[:, 2 * qh_ + c_, :], oT_sb[:])

            from collections import deque
            pending = deque()
            SKEW = 24
            idx = 0
            for qh in range(K):
                for c in range(2):
                    pav_t = psAV.tile([D + 1, 512], F32, tag="av")
                    pavs[(qh, c)] = pav_t
                    for kp in range(NKT // 2):
                        ch = qh * 1024 + c * 512
                        pS = psS.tile([128, 1024], F32, tag="sT")
                        nc.tensor.matmul(pS[:, 0:512], kpair[0:D, kp, :],
                                         qT2[0:D, ch:ch + 512],
                                         start=True, stop=True,
                                         tile_position=(0, 0))
                        nc.tensor.matmul(pS[:, 512:1024],
                                         kpair[D:2 * D, kp, :],
                                         qT2[D:2 * D, ch:ch + 512],
                                         start=True, stop=True,
                                         tile_position=(D, 0))
                        pT = pt.tile([128, 1024], BF, tag="pT")
                        nc.scalar.activation(pT[:], pS[:], AF.Exp, scale=SCALE)
                        pending.append((qh, c, kp, pT))
                        idx += 1
                        if idx > SKEW:
                            n = 2 if len(pending) > 8 else 1
                            for _ in range(min(n, len(pending))):
                                _emit_av(pending.popleft())
            while pending:
                _emit_av(pending.popleft())

            nc.gpsimd.collective_compute(
                "AllToAll", mybir.AluOpType.bypass,
                replica_groups=[list(range(N_CORES))],
                ins=[a2a_in.opt()], outs=[a2a_out.opt()])

            # ---------------- normalize + output projection ----------------
            a2a_out_v = a2a_out.rearrange("(h p) t -> h p t", h=N_CORES)
            den_sb = misc.tile([H, TOK0], F32, tag="den_sb")
            nc.sync.dma_start(den_sb[:], a2a_out_v[:, D, :])
            rdenP = misc.tile([H, TOK0], F32, tag="rdenP")
            nc.vector.reciprocal(rdenP[:], den_sb[:])
            rdenr = misc.tile([H, TOK0], F32R, tag="rdenPr")
            nc.vector.tensor_copy(rdenr[:], rdenP[:])

            onorm = []
            for i in range(4):
                num_sb = misc.tile([128, TOK0], F32, tag="num_sb")
                nc.sync.dma_start(num_sb[:],
                                  a2a_out_v[2 * i:2 * i + 2, 0:D, :])
                prb = psSm.tile([128, TOK0], F32, tag="skf")
                nc.tensor.matmul(prb[:], EBr[:, i * 128:(i + 1) * 128],
                                 rdenr[:], start=True, stop=True)
                on = sb.tile([128, TOK0], BF, tag=f"onorm{i}")
                nc.vector.tensor_mul(on[:], num_sb[:], prb[:])
                onorm.append(on)

            yw = sb.tile([128, 4, C], F32, tag="yw")
            for tt in range(4):
                py = psSm.tile([128, C], F32, tag="skf")
                for ct in range(4):
                    nc.tensor.matmul(py[:],
                                     onorm[ct][:, tt * 128:(tt + 1) * 128],
                                     projTb[ct][:], start=(ct == 0),
                                     stop=False)
                nc.tensor.matmul(py[:], ones1[0:1, 0:128], projBias[:],
                                 start=False, stop=True)
                nc.vector.tensor_copy(yw[:, tt, :], py[:])
            nc.sync.dma_start(y_d.rearrange("(t p) c -> p t c", p=128), yw[:])

    nc.compile()
    return nc


_PROGS = {}


def _get_progs():
    if "l1" not in _PROGS:
        _PROGS["l1"] = _build_l1()
        _PROGS["l2"] = _build_l2()
    return _PROGS["l1"], _PROGS["l2"]


def _run(nc, in_maps, name):
    res = run_bass_kernel_spmd(nc, in_maps, core_ids=list(range(N_CORES)),
                               trace=TRACE)
    if TRACE:
        LAST_EXEC_NS[name] = res.exec_time_ns
    return res.results


def kernel(x, clusters, qkv_w, qkv_b, proj_w, proj_b):
    x = np.asarray(x, np.float32)
    clusters = np.asarray(clusters)
    qkv_w = np.asarray(qkv_w, np.float32)
    qkv_b = np.asarray(qkv_b, np.float32)
    proj_w = np.asarray(proj_w, np.float32)
    proj_b = np.asarray(proj_b, np.float32)
    assert x.shape == (1, N, C)
    # this implementation bakes the arange cluster structure
    assert np.array_equal(clusters.ravel(), np.arange(K * F)), clusters

    l1, l2 = _get_progs()

    # ---- launch 1 host prep ----
    xT = np.concatenate([x[0].T, np.ones((1, N), np.float32)], 0)
    xTb = xT.astype(BF16)
    in1 = []
    for h in range(H):
        wq = qkv_w[h * D:(h + 1) * D, :]            # [64, 512]
        wk = qkv_w[C + h * D: C + (h + 1) * D, :]
        wv = qkv_w[2 * C + h * D: 2 * C + (h + 1) * D, :]
        in1.append({
            "xTb": xTb,
            "wqT": np.ascontiguousarray(wq.T).astype(BF16),
            "wkvT": np.ascontiguousarray(
                np.concatenate([wk, wv], 0).T).astype(BF16),
            "bq": qkv_b[h * D:(h + 1) * D].reshape(D, 1).astype(np.float32),
            "bk": qkv_b[C + h * D:C + (h + 1) * D].reshape(D, 1)
                  .astype(np.float32),
            "bkv": np.concatenate(
                [qkv_b[C + h * D:C + (h + 1) * D],
                 qkv_b[2 * C + h * D:2 * C + (h + 1) * D]])
                .reshape(1, 2 * D).astype(BF16),
        })
    r1 = _run(l1, in1, "l1")

    # ---- host control plane: top-k patch selection ----
    diag = np.stack([r1[c]["diag_d"][0] for c in range(H)])   # [8, 1024]
    s = diag.reshape(H, K, P) * SCALE
    e = np.exp(s - s.max(-1, keepdims=True))
    attn_score = (e / e.sum(-1, keepdims=True)).mean(0)       # [K, P]
    sel = np.sort(np.argsort(-attn_score, kind="stable", axis=-1)[:, :TK], -1)

    # selection matrix: per frame f (= key tile), G[f] is [256, 128] one-hot
    G = np.zeros((16, P, TK), np.float32)
    for f in range(16):
        m = f // F
        G[f, sel[m], np.arange(TK)] = 1.0
    G = G.reshape(N, TK).astype(BF16)
    qselm = np.zeros((K, P), np.float32)
    for m in range(K):
        qselm[m, sel[m]] = 1.0
    qselm = qselm.reshape(NKF, 1)

    projTb = np.concatenate([proj_w.T, proj_b.reshape(1, C)], 0).astype(BF16)
    EB = np.zeros((H, C), np.float32)
    EB[np.arange(C) // D, np.arange(C)] = 1.0
    eye = np.eye(K, dtype=np.float32)

    in2 = []
    for h in range(H):
        in2.append({
            "qT_i": np.asarray(r1[h]["qT_d"]),
            "KV_i": np.asarray(r1[h]["KV_d"]),
            "G_i": G,
            "qselm_i": qselm,
            "projTb_i": projTb,
            "EB_i": EB,
            "eye_i": eye,
        })
    r2 = _run(l2, in2, "l2")

    y = np.concatenate([r2[c]["y_d"] for c in range(N_CORES)], 0)
    return np.ascontiguousarray(y.reshape(1, N, C))
